# revision 10
# baseline (speedup 1.0000x reference)
"""Trainium2 Bass kernel for nn_LuenbergerLDS (B=32, T=2048, N=512, M=512).

Math: the reference is a diagonal complex linear recurrence
    s_t = lam * s_{t-1} + x_t   (per batch, per n; x scalar per t broadcast)
followed by  y = Re(Winv @ s) @ C + x @ D + Do.

Since d == 1 the whole module is a causal LTI SIMO filter:
    y[t, b, m] = sum_{j>=0} H[j, m] * x[t - j, b] + Do[m]
with impulse response (computed on host in float64)
    H[j, m] = sum_n Re(lam_n^j) * A_re[n, m] - Im(lam_n^j) * A_im[n, m]
    A_re = Re(Winv)^T @ C,  A_im = Im(Winv)^T @ C,  H[0] += D.
A window of NLAG*128 = 512 lags truncates at 8.5e-4 of max|y| (measured
exactly on the reference data; gate is 2e-2).

Device work (per core, data-parallel over batch: 4 batches/core): pure
fp8(e4m3) matmuls in DoubleRow perf mode (2 contraction slots per
partition, 0.5 cycles/row -> 2x f32r throughput). For output chunk
t0..t0+127, the stationary operand packs TWO lag tiles' Toeplitz
diagonal slices of a zero-padded, pre-diagonalized x buffer (built on
host, so DMA loads are contiguous); the moving operand packs the two
matching row-flipped H tiles (128x(2*512)). 3 DoubleRow matmuls per
chunk: lags(0,1)_hi, lags(2,3)_hi, and a Dekker compensation pair
(x_lo*H_hi0 + x_hi*H_lo0) that fixes the head tile's fp8 quantization.

Scaling: x ops are x*32, H ops are H*16, so PSUM holds y*512 (absmax
39.3k < fp16 max). Device just downcasts PSUM to fp16 and stores; the
host divides by 512, adds Do, and upcasts to f32. Measured end-to-end
error of this exact pipeline on the reference data: 3.9e-3.
"""

import sys

sys.path.insert(0, "/opt/trn_rl_repo")

import numpy as np
import ml_dtypes

E4 = ml_dtypes.float8_e4m3

# problem dims (hardcoded per harness contract)
B, T, N, M = 32, 2048, 512, 512
NCORES = 8
BLOC = B // NCORES          # batches per core
NLAG = 4                    # lag window = NLAG*128 = 512
MODE = "fp8dr"
SX, SH = 32.0, 16.0         # operand scales; PSUM = y * SX*SH

RPAD = 128 * NLAG - 1       # 511 zero rows ahead of x in xpad
XPLEN = RPAD + T            # 2559
ND = (T + 128 * NLAG - 128) // 128   # 19 diagonal slices (d=0..18)
NZERO = NLAG - 1            # slices d<3 are all zero padding
NREAL = ND - NZERO          # 16 slices carried in DRAM (k=0..15)
TCH = T // 128              # 16 output chunks per batch


def build_program():
    """Build + compile the (SPMD, per-core) Bass program."""
    import concourse.tile as tile
    from concourse import bacc, mybir

    f32 = mybir.dt.float32
    f16 = mybir.dt.float16
    f8 = mybir.dt.float8e4
    DR = mybir.MatmulPerfMode.DoubleRow

    # free-dim element counts of the packed x tensor: [k, kind, col, b]
    # kind: 0 = x_lo, 1 = x_hi; slot k holds diagonal slice d = 18-k
    KSTR = 2 * 128 * BLOC   # 1024 elements per k slot
    nc = bacc.Bacc("TRN2", target_bir_lowering=False, debug=False)
    # DRAM carries all 19 slots (k=16..18 are zeros, host-materialized)
    xall_t = nc.dram_tensor("xall", [128, ND * KSTR], f8, kind="ExternalInput")
    r1_t = nc.dram_tensor("r1", [128, 2 * M], f8, kind="ExternalInput")
    r2_t = nc.dram_tensor("r2", [128, 2 * M], f8, kind="ExternalInput")
    rc_t = nc.dram_tensor("rc", [128, 2 * M], f8, kind="ExternalInput")
    y_t = nc.dram_tensor("y", [BLOC, T, M], f16, kind="ExternalOutput")

    # SBUF granules over k so dependency tracking is fine-grained and a
    # matmul's (k, k+1) pair never crosses a granule: granule g covers
    # k in [LO[g], HI[g]] inclusive, with boundary slots duplicated.
    GLO = [12, 8, 4, 0]
    GHI = [18, 12, 8, 4]

    def gran_of(k):                                # granule for pair (k, k+1)
        for g in range(4):
            if k >= GLO[g]:
                return g, k - GLO[g]
        raise AssertionError

    with tile.TileContext(nc) as tc:
        with (
            tc.tile_pool(name="xs", bufs=1) as xpool,
            tc.tile_pool(name="w", bufs=1) as wpool,
            tc.tile_pool(name="psum", bufs=8, space="PSUM") as psum_pool,
            tc.tile_pool(name="out", bufs=6) as out_pool,
        ):
            load_eng = [nc.sync, nc.scalar, nc.gpsimd]
            # moving H tiles first (small, needed by every matmul)
            r1 = wpool.tile([128, 2 * M], f8, tag="r1")
            nc.sync.dma_start(r1[:], r1_t.ap())
            r2 = wpool.tile([128, 2 * M], f8, tag="r2")
            nc.scalar.dma_start(r2[:], r2_t.ap())
            rc = wpool.tile([128, 2 * M], f8, tag="rc")
            nc.gpsimd.dma_start(rc[:], rc_t.ap())

            # granule tiles + per-slot 128KB loads in consumption order
            xg = [
                xpool.tile([128, (GHI[g] - GLO[g] + 1) * KSTR], f8,
                           tag=f"xg{g}", name=f"xg{g}")
                for g in range(4)
            ]
            # per-slot loads in strict consumption order (tci ascending
            # consumes k descending; tci=0 needs k15..18 + all H tiles)
            li = 0
            for g in range(4):
                nslot = GHI[g] - GLO[g] + 1
                for lo in range(nslot - 1, -1, -1):
                    a = lo * KSTR
                    da = (GLO[g] + lo) * KSTR
                    load_eng[li % 3].dma_start(
                        xg[g][:, a : a + KSTR], xall_t.ap()[:, da : da + KSTR]
                    )
                    li += 1

            xgr = [
                xg[g][:].rearrange(
                    "p (k kind col b) -> p k kind col b",
                    k=GHI[g] - GLO[g] + 1, kind=2, col=128, b=BLOC,
                )
                for g in range(4)
            ]
            r1v = r1[:].rearrange("p (s m) -> p s m", s=2)
            r2v = r2[:].rearrange("p (s m) -> p s m", s=2)
            rcv = rc[:].rearrange("p (s m) -> p s m", s=2)

            store_eng = [nc.sync, nc.scalar, nc.gpsimd]
            gi = 0
            for b in range(BLOC):
                for tci in range(TCH):
                    k1 = (NREAL - 1) - tci          # MM1: lags (0,1)
                    k2 = k1 + 2                     # MM2: lags (2,3)
                    g1, l1 = gran_of(k1)
                    g2, l2 = gran_of(k2)
                    ps = psum_pool.tile([128, M], f32)
                    nc.tensor.matmul(
                        ps[:],
                        lhsT=xgr[g1][:, l1 : l1 + 2, 1, :, b],
                        rhs=r1v,
                        start=True, stop=False, perf_mode=DR,
                    )
                    nc.tensor.matmul(
                        ps[:],
                        lhsT=xgr[g2][:, l2 : l2 + 2, 1, :, b],
                        rhs=r2v,
                        start=False, stop=False, perf_mode=DR,
                    )
                    nc.tensor.matmul(
                        ps[:],
                        lhsT=xgr[g1][:, l1, :, :, b],
                        rhs=rcv,
                        start=False, stop=True, perf_mode=DR,
                    )
                    ot = out_pool.tile([128, M], f16)
                    # PSUM->SBUF fp16 downcast, half on DVE + half on ACT
                    nc.vector.tensor_copy(ot[:, : M // 2], ps[:, : M // 2])
                    nc.scalar.copy(ot[:, M // 2 :], ps[:, M // 2 :])
                    store_eng[gi % 3].dma_start(
                        y_t.ap()[b, 128 * tci : 128 * tci + 128, :], ot[:]
                    )
                    gi += 1

    nc.compile()
    return nc


def _impulse_f64(lnl_re, lnl_im, W_r, W_i, C, D):
    lnl = lnl_re.astype(np.float64) + 1j * lnl_im.astype(np.float64)
    W = W_r.astype(np.float64) + 1j * W_i.astype(np.float64)
    Winv = np.linalg.inv(W)
    A_re = np.ascontiguousarray(Winv.real.T) @ C.astype(np.float64)
    A_im = np.ascontiguousarray(Winv.imag.T) @ C.astype(np.float64)
    j = np.arange(NLAG * 128, dtype=np.float64)
    P = np.exp(np.outer(j, lnl))
    H = P.real @ A_re - P.imag @ A_im
    H[0] += D[0].astype(np.float64)
    return H                                        # (NLAG*128, M) float64


def host_weights(lnl_re, lnl_im, W_r, W_i, C, D, Do):
    """fp8 moving operand tiles r1/r2/rc (hi lag pairs + head Dekker pair)."""
    H = _impulse_f64(lnl_re, lnl_im, W_r, W_i, C, D)
    H_hi8 = (H * SH).astype(np.float32).astype(E4)
    H_lo8 = ((H[:128] * SH).astype(np.float32) - H_hi8[:128].astype(np.float32)).astype(E4)

    def flip(tile8):                                # lag-flip within a tile
        return np.ascontiguousarray(tile8[::-1, :])

    hf = [flip(H_hi8[128 * lg : 128 * (lg + 1)]) for lg in range(NLAG)]
    hfl0 = flip(H_lo8)
    r1 = np.ascontiguousarray(np.concatenate([hf[0], hf[1]], axis=1))
    r2 = np.ascontiguousarray(np.concatenate([hf[2], hf[3]], axis=1))
    rc = np.ascontiguousarray(np.concatenate([hf[0], hfl0], axis=1))
    return {"r1": r1, "r2": r2, "rc": rc}


def make_in_maps(x, weights):
    """Per-core input dict: packed diagonalized hi/lo x + H tiles."""
    x64 = x[:, :, 0].astype(np.float32)             # (B, T)
    xh8 = (x64 * SX).astype(E4)
    xl8 = (x64 * SX - xh8.astype(np.float32)).astype(E4)

    # gather index A[k, p, col] = 128*(18-k) + p + col  into xpad rows
    # (k >= 16 lands entirely in the zero-pad region -> zero slots)
    k = np.arange(ND)
    A = (128 * (ND - 1 - k))[:, None, None] + np.arange(128)[None, :, None] \
        + np.arange(128)[None, None, :]             # (19, 128, 128)

    in_maps = []
    for c in range(NCORES):
        sl = slice(c * BLOC, (c + 1) * BLOC)
        xpad = np.zeros((2, XPLEN, BLOC), E4)       # [kind][row][b]
        xpad[0, RPAD:, :] = xl8[sl].T
        xpad[1, RPAD:, :] = xh8[sl].T
        g = xpad[:, A, :]                           # (2, 19, 128, 128, BLOC)
        xa = np.ascontiguousarray(np.transpose(g, (2, 1, 0, 3, 4)))
        im = dict(weights)
        im["xall"] = xa.reshape(128, ND * 2 * 128 * BLOC)
        in_maps.append(im)
    return in_maps


_prog_cache = {}


def kernel(x, lnl_re, lnl_im, W_r, W_i, C, D, Do):
    from concourse.bass_utils import run_bass_kernel_spmd

    x = np.asarray(x)
    lnl_re, lnl_im = np.asarray(lnl_re), np.asarray(lnl_im)
    W_r, W_i = np.asarray(W_r), np.asarray(W_i)
    C, D, Do = np.asarray(C), np.asarray(D), np.asarray(Do)

    key = (NLAG, MODE)
    if key not in _prog_cache:
        _prog_cache[key] = build_program()
    nc = _prog_cache[key]

    weights = host_weights(lnl_re, lnl_im, W_r, W_i, C, D, Do)
    in_maps = make_in_maps(np.asarray(x, np.float32), weights)
    res = run_bass_kernel_spmd(nc, in_maps, core_ids=list(range(NCORES)))
    y = np.concatenate([res.results[i]["y"] for i in range(NCORES)], axis=0)
    y = y.astype(np.float32) * np.float32(1.0 / (SX * SH)) + Do.astype(np.float32)
    return np.ascontiguousarray(y.astype(np.float32))


# revision 14
# speedup vs baseline: 1.0355x; 1.0355x over previous
"""Trainium2 Bass kernel for nn_LuenbergerLDS (B=32, T=2048, N=512, M=512).

Math: the reference is a diagonal complex linear recurrence
    s_t = lam * s_{t-1} + x_t   (per batch, per n; x scalar per t broadcast)
followed by  y = Re(Winv @ s) @ C + x @ D + Do.

Since d == 1 the whole module is a causal LTI SIMO filter:
    y[t, b, m] = sum_{j>=0} H[j, m] * x[t - j, b] + Do[m]
with impulse response (computed on host in float64)
    H[j, m] = sum_n Re(lam_n^j) * A_re[n, m] - Im(lam_n^j) * A_im[n, m]
    A_re = Re(Winv)^T @ C,  A_im = Im(Winv)^T @ C,  H[0] += D.
A window of NLAG*128 = 512 lags truncates at 8.5e-4 of max|y| (measured
exactly on the reference data; gate is 2e-2).

Device work (per core, data-parallel over batch: 4 batches/core): pure
fp8(e4m3) matmuls in DoubleRow perf mode (2 contraction slots per
partition, 0.5 cycles/row -> 2x f32r throughput). For output chunk
t0..t0+127, the stationary operand packs TWO lag tiles' Toeplitz
diagonal slices of a zero-padded, pre-diagonalized x buffer (built on
host, so DMA loads are contiguous); the moving operand packs the two
matching row-flipped H tiles (128x(2*512)). 3 DoubleRow matmuls per
chunk: lags(0,1)_hi, lags(2,3)_hi, and a Dekker compensation pair
(x_lo*H_hi0 + x_hi*H_lo0) that fixes the head tile's fp8 quantization.

Scaling: x ops are x*32, H ops are H*16, so PSUM holds y*512 (absmax
39.3k < fp16 max). Device just downcasts PSUM to fp16 and stores; the
host divides by 512, adds Do, and upcasts to f32. Measured end-to-end
error of this exact pipeline on the reference data: 3.9e-3.
"""

import sys

sys.path.insert(0, "/opt/trn_rl_repo")

import numpy as np
import ml_dtypes

E4 = ml_dtypes.float8_e4m3

# problem dims (hardcoded per harness contract)
B, T, N, M = 32, 2048, 512, 512
NCORES = 8
BLOC = B // NCORES          # batches per core
NLAG = 4                    # lag window = NLAG*128 = 512
MODE = "fp8dr"
SX, SH = 32.0, 16.0         # operand scales; PSUM = y * SX*SH

RPAD = 128 * NLAG - 1       # 511 zero rows ahead of x in xpad
XPLEN = RPAD + T            # 2559
ND = (T + 128 * NLAG - 128) // 128   # 19 diagonal slices (d=0..18)
NZERO = NLAG - 1            # slices d<3 are all zero padding
NREAL = ND - NZERO          # 16 slices carried in DRAM (k=0..15)
TCH = T // 128              # 16 output chunks per batch


def build_program():
    """Build + compile the (SPMD, per-core) Bass program."""
    import concourse.tile as tile
    from concourse import bacc, mybir

    f32 = mybir.dt.float32
    f16 = mybir.dt.float16
    f8 = mybir.dt.float8e4
    DR = mybir.MatmulPerfMode.DoubleRow

    # free-dim element counts of the packed x tensor: [k, kind, col, b]
    # kind: 0 = x_lo, 1 = x_hi; slot k holds diagonal slice d = 18-k
    KSTR = 2 * 128 * BLOC   # 1024 elements per k slot
    SC = 4                  # output chunks batched per store (4KB DMA lines)
    nc = bacc.Bacc("TRN2", target_bir_lowering=False, debug=False)
    xall_t = nc.dram_tensor("xall", [128, NREAL * KSTR], f8, kind="ExternalInput")
    r1_t = nc.dram_tensor("r1", [128, 2 * M], f8, kind="ExternalInput")
    r2_t = nc.dram_tensor("r2", [128, 2 * M], f8, kind="ExternalInput")
    rc_t = nc.dram_tensor("rc", [128, 2 * M], f8, kind="ExternalInput")
    # y stored chunk-batched: [b, q, p, c*M]; host untransposes
    y_t = nc.dram_tensor(
        "y", [BLOC, TCH // SC, 128, SC * M], f16, kind="ExternalOutput"
    )

    # SBUF granules over k so dependency tracking is fine-grained and a
    # matmul's (k, k+1) pair never crosses a granule: granule g covers
    # k in [GLO[g], GHI[g]] inclusive, boundary slots duplicated.
    # Slices k>15 (d<3) are all-zero: their matmuls are skipped instead.
    GLO = [12, 8, 4, 0]
    GHI = [15, 12, 8, 4]

    def gran_of(k):                                # granule for pair (k, k+1)
        for g in range(4):
            if k >= GLO[g]:
                return g, k - GLO[g]
        raise AssertionError

    with tile.TileContext(nc) as tc:
        with (
            tc.tile_pool(name="xs", bufs=1) as xpool,
            tc.tile_pool(name="w", bufs=1) as wpool,
            tc.tile_pool(name="psum", bufs=8, space="PSUM") as psum_pool,
            tc.tile_pool(name="out", bufs=3) as out_pool,
        ):
            load_eng = [nc.sync, nc.scalar, nc.gpsimd]
            # moving H tiles first (small, needed by every matmul)
            r1 = wpool.tile([128, 2 * M], f8, tag="r1")
            nc.sync.dma_start(r1[:], r1_t.ap())
            r2 = wpool.tile([128, 2 * M], f8, tag="r2")
            nc.scalar.dma_start(r2[:], r2_t.ap())
            rc = wpool.tile([128, 2 * M], f8, tag="rc")
            nc.gpsimd.dma_start(rc[:], rc_t.ap())

            xg = [
                xpool.tile([128, (GHI[g] - GLO[g] + 1) * KSTR], f8,
                           tag=f"xg{g}", name=f"xg{g}")
                for g in range(4)
            ]

            # load list in consumption order: g0 up-front, the rest
            # software-pipelined into the b=0 group loop below
            loads = []
            for g in range(4):
                for lo in range(GHI[g] - GLO[g], -1, -1):
                    loads.append((g, lo))
            li = 0

            def issue_load(n):
                nonlocal li
                for _ in range(n):
                    if li >= len(loads):
                        return
                    g, lo = loads[li]
                    a = lo * KSTR
                    da = (GLO[g] + lo) * KSTR
                    load_eng[li % 3].dma_start(
                        xg[g][:, a : a + KSTR], xall_t.ap()[:, da : da + KSTR]
                    )
                    li += 1

            issue_load(4)                           # g0: k15..k12

            xgr = [
                xg[g][:].rearrange(
                    "p (k kind col b) -> p k kind col b",
                    k=GHI[g] - GLO[g] + 1, kind=2, col=128, b=BLOC,
                )
                for g in range(4)
            ]
            r1v = r1[:].rearrange("p (s m) -> p s m", s=2)
            r2v = r2[:].rearrange("p (s m) -> p s m", s=2)
            rcv = rc[:].rearrange("p (s m) -> p s m", s=2)

            store_eng = [nc.sync, nc.scalar, nc.gpsimd]
            gi = 0
            ot = None
            for b in range(BLOC):
                for tci in range(TCH):
                    if b == 0 and tci < 8:          # prefetch 2 slots/group
                        issue_load(2)
                    k1 = (NREAL - 1) - tci          # MM1: lags (0,1)
                    k2 = k1 + 2                     # MM2: lags (2,3)
                    g1, l1 = gran_of(k1)
                    ps = psum_pool.tile([128, M], f32)
                    if tci == 0:                    # lag1 slice is zero pad
                        nc.tensor.matmul(
                            ps[:], lhsT=xgr[g1][:, l1, 1, :, b],
                            rhs=r1v[:, 0, :], start=True, stop=False,
                        )
                    else:
                        nc.tensor.matmul(
                            ps[:], lhsT=xgr[g1][:, l1 : l1 + 2, 1, :, b],
                            rhs=r1v, start=True, stop=False, perf_mode=DR,
                        )
                    if tci == 2:                    # lag3 slice is zero pad
                        g2, l2 = gran_of(NREAL - 1)
                        nc.tensor.matmul(
                            ps[:], lhsT=xgr[g2][:, l2, 1, :, b],
                            rhs=r2v[:, 0, :], start=False, stop=False,
                        )
                    elif tci > 2:
                        g2, l2 = gran_of(k2)
                        nc.tensor.matmul(
                            ps[:], lhsT=xgr[g2][:, l2 : l2 + 2, 1, :, b],
                            rhs=r2v, start=False, stop=False, perf_mode=DR,
                        )
                    nc.tensor.matmul(
                        ps[:],
                        lhsT=xgr[g1][:, l1, :, :, b],
                        rhs=rcv,
                        start=False, stop=True, perf_mode=DR,
                    )
                    c = tci % SC
                    if c == 0:
                        ot = out_pool.tile([128, SC * M], f16)
                    # PSUM->SBUF fp16 downcast, alternate DVE/ACT
                    if (gi % 2) == 0:
                        nc.vector.tensor_copy(ot[:, c * M : (c + 1) * M], ps[:])
                    else:
                        nc.scalar.copy(ot[:, c * M : (c + 1) * M], ps[:])
                    if c == SC - 1:
                        store_eng[(gi // SC) % 3].dma_start(
                            y_t.ap()[b, tci // SC, :, :], ot[:]
                        )
                    gi += 1

    nc.compile()
    return nc


def _impulse_f64(lnl_re, lnl_im, W_r, W_i, C, D):
    lnl = lnl_re.astype(np.float64) + 1j * lnl_im.astype(np.float64)
    W = W_r.astype(np.float64) + 1j * W_i.astype(np.float64)
    Winv = np.linalg.inv(W)
    A_re = np.ascontiguousarray(Winv.real.T) @ C.astype(np.float64)
    A_im = np.ascontiguousarray(Winv.imag.T) @ C.astype(np.float64)
    j = np.arange(NLAG * 128, dtype=np.float64)
    P = np.exp(np.outer(j, lnl))
    H = P.real @ A_re - P.imag @ A_im
    H[0] += D[0].astype(np.float64)
    return H                                        # (NLAG*128, M) float64


def host_weights(lnl_re, lnl_im, W_r, W_i, C, D, Do):
    """fp8 moving operand tiles r1/r2/rc (hi lag pairs + head Dekker pair)."""
    H = _impulse_f64(lnl_re, lnl_im, W_r, W_i, C, D)
    H_hi8 = (H * SH).astype(np.float32).astype(E4)
    H_lo8 = ((H[:128] * SH).astype(np.float32) - H_hi8[:128].astype(np.float32)).astype(E4)

    def flip(tile8):                                # lag-flip within a tile
        return np.ascontiguousarray(tile8[::-1, :])

    hf = [flip(H_hi8[128 * lg : 128 * (lg + 1)]) for lg in range(NLAG)]
    hfl0 = flip(H_lo8)
    r1 = np.ascontiguousarray(np.concatenate([hf[0], hf[1]], axis=1))
    r2 = np.ascontiguousarray(np.concatenate([hf[2], hf[3]], axis=1))
    rc = np.ascontiguousarray(np.concatenate([hf[0], hfl0], axis=1))
    return {"r1": r1, "r2": r2, "rc": rc}


def make_in_maps(x, weights):
    """Per-core input dict: packed diagonalized hi/lo x + H tiles."""
    x64 = x[:, :, 0].astype(np.float32)             # (B, T)
    xh8 = (x64 * SX).astype(E4)
    xl8 = (x64 * SX - xh8.astype(np.float32)).astype(E4)

    # gather index A[k, p, col] = 128*(18-k) + p + col  into xpad rows
    k = np.arange(NREAL)
    A = (128 * (ND - 1 - k))[:, None, None] + np.arange(128)[None, :, None] \
        + np.arange(128)[None, None, :]             # (16, 128, 128)

    in_maps = []
    for c in range(NCORES):
        sl = slice(c * BLOC, (c + 1) * BLOC)
        xpad = np.zeros((2, XPLEN, BLOC), E4)       # [kind][row][b]
        xpad[0, RPAD:, :] = xl8[sl].T
        xpad[1, RPAD:, :] = xh8[sl].T
        g = xpad[:, A, :]                           # (2, 16, 128, 128, BLOC)
        xa = np.ascontiguousarray(np.transpose(g, (2, 1, 0, 3, 4)))
        im = dict(weights)
        im["xall"] = xa.reshape(128, NREAL * 2 * 128 * BLOC)
        in_maps.append(im)
    return in_maps


_prog_cache = {}


def kernel(x, lnl_re, lnl_im, W_r, W_i, C, D, Do):
    from concourse.bass_utils import run_bass_kernel_spmd

    x = np.asarray(x)
    lnl_re, lnl_im = np.asarray(lnl_re), np.asarray(lnl_im)
    W_r, W_i = np.asarray(W_r), np.asarray(W_i)
    C, D, Do = np.asarray(C), np.asarray(D), np.asarray(Do)

    key = (NLAG, MODE)
    if key not in _prog_cache:
        _prog_cache[key] = build_program()
    nc = _prog_cache[key]

    weights = host_weights(lnl_re, lnl_im, W_r, W_i, C, D, Do)
    in_maps = make_in_maps(np.asarray(x, np.float32), weights)
    res = run_bass_kernel_spmd(nc, in_maps, core_ids=list(range(NCORES)))
    # device layout [bloc, q, p, c*M]: t = 512*q + 128*c + p
    y = np.concatenate([res.results[i]["y"] for i in range(NCORES)], axis=0)
    y = y.reshape(B, TCH // 4, 128, 4, M).transpose(0, 1, 3, 2, 4).reshape(B, T, M)
    y = y.astype(np.float32) * np.float32(1.0 / (SX * SH)) + Do.astype(np.float32)
    return np.ascontiguousarray(y.astype(np.float32))
